# revision 8
# baseline (speedup 1.0000x reference)
"""Trainium2 Bass kernel for nn_Decoder (autoregressive GRU decoder with skip
connections).

Strategy:
  - Data-parallel over batch: B=128 -> 8 cores x 16. No collectives; every
    core runs an identical program on its own batch slice.
  - The `input` tensor's VALUES are unused by the reference (the GRU consumes
    its own previous projected output), so it is never transferred.
  - mask0/mask1/skip_size are host-visible at build time; the T=512 step loop
    is fully unrolled and specialized per step (the masked adds become tile
    aliasing / a single add / a zero tile).
  - Layout: states are feature-on-partition, fp16. A hidden-state slot is
    [128 partitions, 2*16] (H=256 split into 2 chunks of 128, batch=16 on the
    free axis). The full GRU-output history buf lives in SBUF ([128, 512*32]
    fp16 = 32KB/partition), as does the projected-output history.
  - Matmuls: weights stationary (fp16 => fast-weight-load), states moving
    (N=16). gi and gh accumulate into the same PSUM tile so ir+hr / iz+hz
    need no extra adds. n-gate keeps inn and hn in separate PSUM columns.
  - Output projection feeds both the output history and the next step's GRU
    input, so it is computed once per step into [X=64, B] layout; the final
    [B, T, X] transpose is done at the end with 64 PE-transposes.
"""

import os
import numpy as np

B, T, X, H = 128, 512, 64, 256
N_CORES = 8
BL = B // N_CORES  # 16 batch per core
NG = 6             # gate chunks of 128 (r0 r1 z0 z1 n0 n1)


def _skip_tables(T, skip):
    # replicates the reference's Python-list negative indexing
    g_idx = np.zeros(T, np.int64); g_zero = np.zeros(T, bool)
    p_idx = np.zeros(T, np.int64); p_zero = np.zeros(T, bool)
    for i in range(T):
        pg = 2 * i if i < skip else i - skip
        pp = 2 * i + 1 if i < skip else i - skip
        g_zero[i] = pg < skip; g_idx[i] = max(pg - skip, 0)
        p_zero[i] = pp < skip; p_idx[i] = max(pp - skip, 0)
    return g_idx, g_zero, p_idx, p_zero


def _build_program(n_steps, a0, a1, g_idx, g_zero, p_idx, p_zero,
                   rz_bias_nz, inn_bias_nz, hn_bias_nz, out_bias_nz):
    import concourse.bass as bass  # noqa: F401
    import concourse.bacc as bacc
    import concourse.tile as tile
    from concourse import mybir
    from contextlib import ExitStack

    f16 = mybir.dt.float16
    f32 = mybir.dt.float32
    AF = mybir.ActivationFunctionType
    OP = mybir.AluOpType

    any_bias = rz_bias_nz or inn_bias_nz or hn_bias_nz

    nc = bacc.Bacc("TRN2", target_bir_lowering=False, debug=False)

    henc_d = nc.dram_tensor("henc", [128, 2 * BL], f16, kind="ExternalInput")
    wih_d = nc.dram_tensor("wih", [64, 768], f16, kind="ExternalInput")
    whh_d = nc.dram_tensor("whh", [128, 2 * 768], f16, kind="ExternalInput")
    wout_d = nc.dram_tensor("wout", [128, 2 * 64], f16, kind="ExternalInput")
    ident_d = nc.dram_tensor("ident", [64, 64], f16, kind="ExternalInput")
    bias_d = nc.dram_tensor("biases", [1, 1024], f16, kind="ExternalInput")
    bout_d = nc.dram_tensor("bout", [64, 1], f32, kind="ExternalInput")
    out_d = nc.dram_tensor("out", [BL, T, X], f32, kind="ExternalOutput")

    with tile.TileContext(nc) as tc, ExitStack() as ctx:
        const = ctx.enter_context(tc.tile_pool(name="const", bufs=1))
        big = ctx.enter_context(tc.tile_pool(name="big", bufs=1))

        wih = const.tile([64, 768], f16)
        nc.sync.dma_start(wih[:, :], wih_d[:, :])
        whh = const.tile([128, 2 * 768], f16)
        nc.sync.dma_start(whh[:, :], whh_d[:, :])
        wout = const.tile([128, 2 * 64], f16)
        nc.sync.dma_start(wout[:, :], wout_d[:, :])
        ident = const.tile([64, 64], f16)
        nc.sync.dma_start(ident[:, :], ident_d[:, :])
        henc = const.tile([128, 2 * BL], f16)
        nc.sync.dma_start(henc[:, :], henc_d[:, :])
        zero = const.tile([128, 2 * BL], f16)
        nc.vector.memset(zero[:, :], 0.0)
        bias = const.tile([1, 1024], f16)
        ones = const.tile([1, BL], f16)
        bout = const.tile([64, 1], f32)
        if any_bias:
            nc.sync.dma_start(bias[:, :], bias_d[:, :])
            nc.vector.memset(ones[:, :], 1.0)
        if out_bias_nz:
            nc.sync.dma_start(bout[:, :], bout_d[:, :])

        # GRU output history: slot i at cols [i*32, (i+1)*32); chunk c at
        # [i*32 + c*16, ...). Also the h state itself (h_{i} == slot i).
        buf = big.tile([128, T * 2 * BL], f16)
        # projected-output history: slot t holds x_t (the GRU input at step
        # t); slot t+1 is written by step t's projection. Slot 0 is the GO
        # token (zeros).
        xh = big.tile([64, (T + 1) * BL], f16)
        nc.vector.memset(xh[:, 0:BL], 0.0)

        def slot(i):
            if i < 0:
                return henc[:, :]
            return buf[:, i * 2 * BL:(i + 1) * 2 * BL]

        with (
            tc.tile_pool(name="work", bufs=8) as work,
            tc.tile_pool(name="psum", bufs=3, space="PSUM") as psum,
            tc.tile_pool(name="psumx", bufs=2, space="PSUM") as psumx,
        ):
            for t in range(n_steps):
                # ---- resolve hidden = a0*h_prev + a1*skip_g ----
                terms = []
                if a0[t]:
                    terms.append(t - 1)
                if a1[t] and not g_zero[t] and g_idx[t] < t:
                    terms.append(int(g_idx[t]))
                if len(terms) == 0:
                    hid = zero[:, :]
                    hid_zero = True
                elif len(terms) == 1:
                    hid = slot(terms[0])
                    hid_zero = False
                else:
                    hidt = work.tile([128, 2 * BL], f16, tag="hid")
                    nc.vector.tensor_add(hidt[:, :], slot(terms[0]), slot(terms[1]))
                    hid = hidt[:, :]
                    hid_zero = False

                has_gi = t > 0
                has_gh = not hid_zero

                if not has_gi and not has_gh and not any_bias:
                    # gates see all-zero inputs: r=z=0.5, n=tanh(0)=0
                    # => h_new = 0.5*0 + 0.5*0 = 0
                    nc.vector.memset(slot(t), 0.0)
                else:
                    # ---- gate matmuls ----
                    # rz psum: cols [g*16,(g+1)*16) for g in r0 r1 z0 z1
                    ps_rz = psum.tile([128, 4 * BL], f32, tag="rz")
                    # n psum: cols 0:32 = inn chunks, 32:64 = hn chunks
                    ps_n = psum.tile([128, 4 * BL], f32, tag="n")
                    xrhs = xh[:, t * BL:(t + 1) * BL]

                    def gate_group(out_ap, gc, with_gi, with_gh, bias_col):
                        mms = []
                        if with_gh:
                            mms.append(("gh", 0))
                            mms.append(("gh", 1))
                        if with_gi:
                            mms.append(("gi", 0))
                        if bias_col is not None:
                            mms.append(("bias", bias_col))
                        if not mms:
                            nc.vector.memset(out_ap, 0.0)
                            return
                        for j, (kind, k) in enumerate(mms):
                            st = j == 0
                            sp = j == len(mms) - 1
                            if kind == "gh":
                                nc.tensor.matmul(
                                    out_ap,
                                    whh[:, k * 768 + gc * 128:k * 768 + (gc + 1) * 128],
                                    hid[:, k * BL:(k + 1) * BL],
                                    start=st, stop=sp)
                            elif kind == "gi":
                                nc.tensor.matmul(
                                    out_ap,
                                    wih[:, gc * 128:(gc + 1) * 128],
                                    xrhs,
                                    start=st, stop=sp)
                            else:
                                nc.tensor.matmul(
                                    out_ap,
                                    bias[:, k:k + 128],
                                    ones[:, :],
                                    start=st, stop=sp)

                    # critical-path-first PE order: r gates, hn, inn, then z
                    for g in range(2):  # r0 r1
                        bc = g * 128 if rz_bias_nz else None
                        gate_group(ps_rz[:, g * BL:(g + 1) * BL], g, has_gi,
                                   has_gh, bc)
                    if has_gh or hn_bias_nz:
                        for c in range(2):  # hn chunks (gh only)
                            bc = 768 + c * 128 if hn_bias_nz else None
                            gate_group(ps_n[:, (2 + c) * BL:(3 + c) * BL],
                                       4 + c, False, has_gh, bc)
                    if has_gi or inn_bias_nz:
                        for c in range(2):  # inn chunks (gi only)
                            bc = 512 + c * 128 if inn_bias_nz else None
                            gate_group(ps_n[:, c * BL:(c + 1) * BL], 4 + c,
                                       has_gi, False, bc)
                    for g in range(2, 4):  # z0 z1
                        bc = g * 128 if rz_bias_nz else None
                        gate_group(ps_rz[:, g * BL:(g + 1) * BL], g, has_gi,
                                   has_gh, bc)

                    # ---- gates ----
                    rz = work.tile([128, 4 * BL], f32, tag="rz_sb")
                    r_ap = rz[:, 0:2 * BL]
                    z_ap = rz[:, 2 * BL:4 * BL]
                    # r sigmoid is on the critical path; z is hidden under
                    # the n-path
                    nc.scalar.activation(r_ap, ps_rz[:, 0:2 * BL], AF.Sigmoid)
                    nc.scalar.activation(z_ap, ps_rz[:, 2 * BL:4 * BL],
                                         AF.Sigmoid)

                    # off-critical-path work on gpsimd
                    omz = work.tile([128, 2 * BL], f16, tag="omz")
                    nc.gpsimd.tensor_scalar(omz[:, :], z_ap, -1.0, 1.0,
                                            OP.mult, OP.add)
                    if not hid_zero:
                        zh = work.tile([128, 2 * BL], f16, tag="zh")
                        nc.gpsimd.tensor_mul(zh[:, :], z_ap, hid)

                    # n = tanh(inn + r*hn)
                    if has_gh or hn_bias_nz:
                        rhn = work.tile([128, 2 * BL], f32, tag="rhn")
                        nc.vector.tensor_mul(rhn[:, :], r_ap,
                                             ps_n[:, 2 * BL:4 * BL])
                        if has_gi or inn_bias_nz:
                            npre = work.tile([128, 2 * BL], f32, tag="npre")
                            nc.vector.tensor_add(npre[:, :], rhn[:, :],
                                                 ps_n[:, 0:2 * BL])
                            n_src = npre[:, :]
                        else:
                            n_src = rhn[:, :]
                    else:
                        n_src = ps_n[:, 0:2 * BL]
                    n_sb = work.tile([128, 2 * BL], f16, tag="n_sb")
                    nc.scalar.activation(n_sb[:, :], n_src, AF.Tanh)

                    # ---- h_new = (1-z)*n + z*hidden -> buf[t] ----
                    if hid_zero:
                        nc.vector.tensor_mul(slot(t), omz[:, :], n_sb[:, :])
                    else:
                        t1 = work.tile([128, 2 * BL], f16, tag="t1")
                        nc.vector.tensor_mul(t1[:, :], omz[:, :], n_sb[:, :])
                        nc.vector.tensor_add(slot(t), t1[:, :], zh[:, :])

                # ---- projection out_t = (h_prev + skip_p) @ W_out^T ----
                pterms = [t - 1]
                if not p_zero[t] and p_idx[t] <= t:
                    pterms.append(int(p_idx[t]))
                if len(pterms) == 1:
                    tmp = slot(pterms[0])
                else:
                    tmpt = work.tile([128, 2 * BL], f16, tag="tmp")
                    nc.gpsimd.tensor_add(tmpt[:, :], slot(pterms[0]),
                                         slot(pterms[1]))
                    tmp = tmpt[:, :]
                ps_x = psumx.tile([64, BL], f32, tag="xp")
                nc.tensor.matmul(ps_x[:, :], wout[:, 0:64], tmp[:, 0:BL],
                                 start=True, stop=False)
                nc.tensor.matmul(ps_x[:, :], wout[:, 64:128], tmp[:, BL:2 * BL],
                                 start=False, stop=True)
                xdst = xh[:, (t + 1) * BL:(t + 2) * BL]
                if out_bias_nz:
                    nc.scalar.activation(xdst, ps_x[:, :], AF.Identity,
                                         bias=bout[:, :])
                else:
                    nc.vector.tensor_copy(xdst, ps_x[:, :])

        # ---- end phase: transpose xh slots 1..n_steps into out [BL, T, X] ----
        xh3 = xh[:, :].rearrange("p (s b) -> p s b", b=BL)
        n_tc = (n_steps + 127) // 128
        with (
            tc.tile_pool(name="endsb", bufs=2) as endsb,
            tc.tile_pool(name="endps", bufs=4, space="PSUM") as endps,
        ):
            for bt in range(BL):
                sbv = endsb.tile([128, n_tc * 64], f32, tag="osb")
                for c in range(n_tc):
                    pst = endps.tile([128, 64], f16, tag="pst")
                    lo = 1 + c * 128
                    nsl = min(128, n_steps - c * 128)
                    nc.tensor.transpose(pst[0:nsl, :],
                                        xh3[:, lo:lo + nsl, bt], ident[:, :])
                    nc.scalar.activation(sbv[0:nsl, c * 64:(c + 1) * 64],
                                         pst[0:nsl, :], AF.Copy)
                for c in range(n_tc):
                    nsl = min(128, n_steps - c * 128)
                    nc.sync.dma_start(
                        out=out_d[bt, c * 128:c * 128 + nsl, :],
                        in_=sbv[0:nsl, c * 64:(c + 1) * 64])
    nc.compile()
    return nc


_PROG_CACHE = {}


def _get_program(key, *args):
    if key not in _PROG_CACHE:
        _PROG_CACHE[key] = _build_program(*args)
    return _PROG_CACHE[key]


def kernel_with_results(input, h_enc, W_ih, W_hh, b_ih, b_hh, W_out, b_out,
                        mask0, mask1, skip_size, trace=False):
    from concourse.bass_utils import run_bass_kernel_spmd

    skip = int(skip_size)
    g_idx, g_zero, p_idx, p_zero = _skip_tables(T, skip)
    a0 = np.asarray(mask0).astype(np.int64) != 0
    a1 = np.asarray(mask1).astype(np.int64) != 0

    b_ih = np.asarray(b_ih, np.float32)
    b_hh = np.asarray(b_hh, np.float32)
    b_out = np.asarray(b_out, np.float32)
    rz_bias = (b_ih + b_hh)[0:512]
    inn_bias = b_ih[512:768]
    hn_bias = b_hh[512:768]
    rz_nz = bool(np.any(rz_bias != 0))
    inn_nz = bool(np.any(inn_bias != 0))
    hn_nz = bool(np.any(hn_bias != 0))
    bout_nz = bool(np.any(b_out != 0))

    key = (T, a0.tobytes(), a1.tobytes(), skip, rz_nz, inn_nz, hn_nz, bout_nz)
    nc = _get_program(key, T, a0, a1, g_idx, g_zero, p_idx, p_zero,
                      rz_nz, inn_nz, hn_nz, bout_nz)

    W_ih = np.asarray(W_ih, np.float32)
    W_hh = np.asarray(W_hh, np.float32)
    W_out = np.asarray(W_out, np.float32)
    h_enc = np.asarray(h_enc, np.float32)

    wih_h = np.ascontiguousarray(W_ih.T).astype(np.float16)          # [64, 768]
    whhT = W_hh.T.astype(np.float16)                                  # [256, 768]
    whh_h = np.ascontiguousarray(np.concatenate(
        [whhT[0:128, :], whhT[128:256, :]], axis=1))                  # [128, 1536]
    woutT = W_out.T.astype(np.float16)                                # [256, 64]
    wout_h = np.ascontiguousarray(np.concatenate(
        [woutT[0:128, :], woutT[128:256, :]], axis=1))                # [128, 128]
    ident_h = np.eye(64, dtype=np.float16)
    bias_h = np.zeros((1, 1024), np.float16)
    bias_h[0, 0:512] = rz_bias.astype(np.float16)
    bias_h[0, 512:768] = inn_bias.astype(np.float16)
    bias_h[0, 768:1024] = hn_bias.astype(np.float16)
    bout_h = b_out.reshape(64, 1).astype(np.float32)

    in_maps = []
    for c in range(N_CORES):
        hc = h_enc[c * BL:(c + 1) * BL]                               # [16, 256]
        henc_h = np.ascontiguousarray(
            hc.T.reshape(2, 128, BL).transpose(1, 0, 2).reshape(128, 2 * BL)
        ).astype(np.float16)
        in_maps.append({
            "henc": henc_h,
            "wih": wih_h,
            "whh": whh_h,
            "wout": wout_h,
            "ident": ident_h,
            "biases": bias_h,
            "bout": bout_h,
        })

    res = run_bass_kernel_spmd(nc, in_maps, core_ids=list(range(N_CORES)),
                               trace=trace)
    out = np.concatenate([r["out"] for r in res.results], axis=0)
    return out.astype(np.float32), res


def kernel(**inputs):
    out, _ = kernel_with_results(**inputs)
    return out


# revision 14
# speedup vs baseline: 1.1430x; 1.1430x over previous
"""Trainium2 Bass kernel for nn_Decoder (autoregressive GRU decoder with skip
connections).

Strategy:
  - Data-parallel over batch: B=128 -> 8 cores x 16. No collectives; every
    core runs an identical program on its own batch slice.
  - The `input` tensor's VALUES are unused by the reference (the GRU consumes
    its own previous projected output), so it is never transferred.
  - mask0/mask1/skip_size are host-visible at build time; the T=512 step loop
    is fully unrolled and specialized per step (the masked adds become tile
    aliasing / a single add / a zero tile).
  - Layout: states are feature-on-partition, fp16. A hidden-state slot is
    [128 partitions, 2*16] (H=256 split into 2 chunks of 128, batch=16 on the
    free axis). The full GRU-output history buf lives in SBUF ([128, 512*32]
    fp16 = 32KB/partition), as does the projected-output history.
  - Matmuls: weights stationary (fp16 => fast-weight-load), states moving
    (N=16). gi and gh accumulate into the same PSUM tile so ir+hr / iz+hz
    need no extra adds. n-gate keeps inn and hn in separate PSUM columns.
  - Output projection feeds both the output history and the next step's GRU
    input, so it is computed once per step into [X=64, B] layout; the final
    [B, T, X] transpose is done at the end with 64 PE-transposes.
"""

import os
import numpy as np

B, T, X, H = 128, 512, 64, 256
N_CORES = 8
BL = B // N_CORES  # 16 batch per core
NG = 6             # gate chunks of 128 (r0 r1 z0 z1 n0 n1)


def _skip_tables(T, skip):
    # replicates the reference's Python-list negative indexing
    g_idx = np.zeros(T, np.int64); g_zero = np.zeros(T, bool)
    p_idx = np.zeros(T, np.int64); p_zero = np.zeros(T, bool)
    for i in range(T):
        pg = 2 * i if i < skip else i - skip
        pp = 2 * i + 1 if i < skip else i - skip
        g_zero[i] = pg < skip; g_idx[i] = max(pg - skip, 0)
        p_zero[i] = pp < skip; p_idx[i] = max(pp - skip, 0)
    return g_idx, g_zero, p_idx, p_zero


def _build_program(n_steps, a0, a1, g_idx, g_zero, p_idx, p_zero,
                   rz_bias_nz, inn_bias_nz, hn_bias_nz, out_bias_nz,
                   use_bf16=False):
    import concourse.bass as bass  # noqa: F401
    import concourse.bacc as bacc
    import concourse.tile as tile
    from concourse import mybir
    from contextlib import ExitStack

    f16 = mybir.dt.bfloat16 if use_bf16 else mybir.dt.float16
    f32 = mybir.dt.float32
    AF = mybir.ActivationFunctionType
    OP = mybir.AluOpType

    any_bias = rz_bias_nz or inn_bias_nz or hn_bias_nz

    nc = bacc.Bacc("TRN2", target_bir_lowering=False, debug=False)

    henc_d = nc.dram_tensor("henc", [128, 2 * BL], f16, kind="ExternalInput")
    wih_d = nc.dram_tensor("wih", [64, 768], f16, kind="ExternalInput")
    whh_d = nc.dram_tensor("whh", [128, 2 * 768], f16, kind="ExternalInput")
    wout_d = nc.dram_tensor("wout", [128, 2 * 64], f16, kind="ExternalInput")
    ident_d = nc.dram_tensor("ident", [64, 64], f16, kind="ExternalInput")
    bias_d = nc.dram_tensor("biases", [1, 1024], f16, kind="ExternalInput")
    bout_d = nc.dram_tensor("bout", [64, 1], f32, kind="ExternalInput")
    out_d = nc.dram_tensor("out", [BL, T, X], f32, kind="ExternalOutput")

    with tile.TileContext(nc) as tc, ExitStack() as ctx:
        const = ctx.enter_context(tc.tile_pool(name="const", bufs=1))
        big = ctx.enter_context(tc.tile_pool(name="big", bufs=1))

        wih = const.tile([64, 768], f16)
        nc.sync.dma_start(wih[:, :], wih_d[:, :])
        whh = const.tile([128, 2 * 768], f16)
        nc.sync.dma_start(whh[:, :], whh_d[:, :])
        wout = const.tile([128, 2 * 64], f16)
        nc.sync.dma_start(wout[:, :], wout_d[:, :])
        ident = const.tile([64, 64], f16)
        nc.sync.dma_start(ident[:, :], ident_d[:, :])
        henc = const.tile([128, 2 * BL], f16)
        nc.sync.dma_start(henc[:, :], henc_d[:, :])
        zero = const.tile([128, 2 * BL], f16)
        nc.vector.memset(zero[:, :], 0.0)
        bias = const.tile([1, 1024], f16)
        ones = const.tile([1, BL], f16)
        bout = const.tile([64, 1], f32)
        if any_bias:
            nc.sync.dma_start(bias[:, :], bias_d[:, :])
            nc.vector.memset(ones[:, :], 1.0)
        if out_bias_nz:
            nc.sync.dma_start(bout[:, :], bout_d[:, :])

        # GRU output history: slot i at cols [i*32, (i+1)*32); chunk c at
        # [i*32 + c*16, ...). Also the h state itself (h_{i} == slot i).
        buf = big.tile([128, T * 2 * BL], f16)
        # projected-output history: slot t holds x_t (the GRU input at step
        # t); slot t+1 is written by step t's projection. Slot 0 is the GO
        # token (zeros).
        xh = big.tile([64, (T + 1) * BL], f16)
        nc.vector.memset(xh[:, 0:BL], 0.0)

        def slot(i):
            if i < 0:
                return henc[:, :]
            return buf[:, i * 2 * BL:(i + 1) * 2 * BL]

        with (
            tc.tile_pool(name="work", bufs=8) as work,
            tc.tile_pool(name="psum", bufs=2, space="PSUM") as psum,
            tc.tile_pool(name="psumx", bufs=2, space="PSUM") as psumx,
        ):
            for t in range(n_steps):
                # ---- resolve hidden = a0*h_prev + a1*skip_g ----
                terms = []
                if a0[t]:
                    terms.append(t - 1)
                if a1[t] and not g_zero[t] and g_idx[t] < t:
                    terms.append(int(g_idx[t]))
                if len(terms) == 0:
                    hid = zero[:, :]
                    hid_zero = True
                elif len(terms) == 1:
                    hid = slot(terms[0])
                    hid_zero = False
                else:
                    hidt = work.tile([128, 2 * BL], f16, tag="hid")
                    nc.vector.tensor_add(hidt[:, :], slot(terms[0]), slot(terms[1]))
                    hid = hidt[:, :]
                    hid_zero = False

                has_gi = t > 0
                has_gh = not hid_zero

                if not has_gi and not has_gh and not any_bias:
                    # gates see all-zero inputs: r=z=0.5, n=tanh(0)=0
                    # => h_new = 0.5*0 + 0.5*0 = 0
                    nc.vector.memset(slot(t), 0.0)
                else:
                    # ---- gate matmuls ----
                    # separate PSUM tiles (= banks) for r / z / n so the
                    # r-sigmoid is not bank-serialized behind z/proj writes
                    ps_r = psum.tile([128, 2 * BL], f32, tag="r")
                    ps_z = psum.tile([128, 2 * BL], f32, tag="z")
                    # n psum: cols 0:32 = inn chunks, 32:64 = hn chunks
                    ps_n = psum.tile([128, 4 * BL], f32, tag="n")
                    xrhs = xh[:, t * BL:(t + 1) * BL]

                    def gate_group(out_ap, gc, with_gi, with_gh, bias_col):
                        mms = []
                        if with_gh:
                            mms.append(("gh", 0))
                            mms.append(("gh", 1))
                        if with_gi:
                            mms.append(("gi", 0))
                        if bias_col is not None:
                            mms.append(("bias", bias_col))
                        if not mms:
                            nc.vector.memset(out_ap, 0.0)
                            return
                        for j, (kind, k) in enumerate(mms):
                            st = j == 0
                            sp = j == len(mms) - 1
                            if kind == "gh":
                                nc.tensor.matmul(
                                    out_ap,
                                    whh[:, k * 768 + gc * 128:k * 768 + (gc + 1) * 128],
                                    hid[:, k * BL:(k + 1) * BL],
                                    start=st, stop=sp)
                            elif kind == "gi":
                                nc.tensor.matmul(
                                    out_ap,
                                    wih[:, gc * 128:(gc + 1) * 128],
                                    xrhs,
                                    start=st, stop=sp)
                            else:
                                nc.tensor.matmul(
                                    out_ap,
                                    bias[:, k:k + 128],
                                    ones[:, :],
                                    start=st, stop=sp)

                    # critical-path-first PE order: r gates, hn, inn, then z
                    for g in range(2):  # r0 r1
                        bc = g * 128 if rz_bias_nz else None
                        gate_group(ps_r[:, g * BL:(g + 1) * BL], g, has_gi,
                                   has_gh, bc)
                    if has_gh or hn_bias_nz:
                        for c in range(2):  # hn chunks (gh only)
                            bc = 768 + c * 128 if hn_bias_nz else None
                            gate_group(ps_n[:, (2 + c) * BL:(3 + c) * BL],
                                       4 + c, False, has_gh, bc)
                    if has_gi or inn_bias_nz:
                        for c in range(2):  # inn chunks (gi only)
                            bc = 512 + c * 128 if inn_bias_nz else None
                            gate_group(ps_n[:, c * BL:(c + 1) * BL], 4 + c,
                                       has_gi, False, bc)
                    for g in range(2, 4):  # z0 z1
                        bc = g * 128 if rz_bias_nz else None
                        gate_group(ps_z[:, (g - 2) * BL:(g - 1) * BL], g,
                                   has_gi, has_gh, bc)

                    # ---- gates ----
                    rz = work.tile([128, 4 * BL], f32, tag="rz_sb")
                    r_ap = rz[:, 0:2 * BL]
                    z_ap = rz[:, 2 * BL:4 * BL]
                    # r sigmoid is on the critical path; z is hidden under
                    # the n-path
                    nc.scalar.activation(r_ap, ps_r[:, :], AF.Sigmoid)
                    nc.scalar.activation(z_ap, ps_z[:, :], AF.Sigmoid)

                    # off-critical-path work on gpsimd
                    omz = work.tile([128, 2 * BL], f16, tag="omz")
                    nc.gpsimd.tensor_scalar(omz[:, :], z_ap, -1.0, 1.0,
                                            OP.mult, OP.add)
                    if not hid_zero:
                        zh = work.tile([128, 2 * BL], f16, tag="zh")
                        nc.gpsimd.tensor_mul(zh[:, :], z_ap, hid)

                    # n = tanh(inn + r*hn)
                    if has_gh or hn_bias_nz:
                        rhn = work.tile([128, 2 * BL], f32, tag="rhn")
                        nc.vector.tensor_mul(rhn[:, :], r_ap,
                                             ps_n[:, 2 * BL:4 * BL])
                        if has_gi or inn_bias_nz:
                            npre = work.tile([128, 2 * BL], f32, tag="npre")
                            nc.vector.tensor_add(npre[:, :], rhn[:, :],
                                                 ps_n[:, 0:2 * BL])
                            n_src = npre[:, :]
                        else:
                            n_src = rhn[:, :]
                    else:
                        n_src = ps_n[:, 0:2 * BL]
                    n_sb = work.tile([128, 2 * BL], f16, tag="n_sb")
                    nc.scalar.activation(n_sb[:, :], n_src, AF.Tanh)

                    # ---- h_new = (1-z)*n + z*hidden -> buf[t] ----
                    if hid_zero:
                        nc.vector.tensor_mul(slot(t), omz[:, :], n_sb[:, :])
                    else:
                        t1 = work.tile([128, 2 * BL], f16, tag="t1")
                        nc.vector.tensor_mul(t1[:, :], omz[:, :], n_sb[:, :])
                        nc.vector.tensor_add(slot(t), t1[:, :], zh[:, :])

                # ---- projection out_t = (h_prev + skip_p) @ W_out^T ----
                # projection input (h_prev + skip_p) is fed as multiple
                # moving operands accumulated in PSUM - no explicit add
                pterms = [t - 1]
                if not p_zero[t] and p_idx[t] <= t:
                    pterms.append(int(p_idx[t]))
                ps_x = psumx.tile([64, BL], f32, tag="xp")
                n_mm = 2 * len(pterms)
                j = 0
                for pt in pterms:
                    src = slot(pt)
                    for k in range(2):
                        nc.tensor.matmul(ps_x[:, :],
                                         wout[:, k * 64:(k + 1) * 64],
                                         src[:, k * BL:(k + 1) * BL],
                                         start=(j == 0), stop=(j == n_mm - 1))
                        j += 1
                xdst = xh[:, (t + 1) * BL:(t + 2) * BL]
                if out_bias_nz:
                    nc.scalar.activation(xdst, ps_x[:, :], AF.Identity,
                                         bias=bout[:, :])
                else:
                    nc.vector.tensor_copy(xdst, ps_x[:, :])

        # ---- end phase: transpose xh slots 1..n_steps into out [BL, T, X] ----
        xh3 = xh[:, :].rearrange("p (s b) -> p s b", b=BL)
        n_tc = (n_steps + 127) // 128
        with (
            tc.tile_pool(name="endsb", bufs=2) as endsb,
            tc.tile_pool(name="endps", bufs=4, space="PSUM") as endps,
        ):
            for bt in range(BL):
                sbv = endsb.tile([128, n_tc * 64], f32, tag="osb")
                for c in range(n_tc):
                    pst = endps.tile([128, 64], f16, tag="pst")
                    lo = 1 + c * 128
                    nsl = min(128, n_steps - c * 128)
                    nc.tensor.transpose(pst[0:nsl, :],
                                        xh3[:, lo:lo + nsl, bt], ident[:, :])
                    nc.scalar.activation(sbv[0:nsl, c * 64:(c + 1) * 64],
                                         pst[0:nsl, :], AF.Copy)
                for c in range(n_tc):
                    nsl = min(128, n_steps - c * 128)
                    nc.sync.dma_start(
                        out=out_d[bt, c * 128:c * 128 + nsl, :],
                        in_=sbv[0:nsl, c * 64:(c + 1) * 64])
    nc.compile()
    return nc


_PROG_CACHE = {}


def _get_program(key, *args):
    if key not in _PROG_CACHE:
        _PROG_CACHE[key] = _build_program(*args)
    return _PROG_CACHE[key]


def kernel_with_results(input, h_enc, W_ih, W_hh, b_ih, b_hh, W_out, b_out,
                        mask0, mask1, skip_size, trace=False):
    from concourse.bass_utils import run_bass_kernel_spmd
    import ml_dtypes

    USE_BF16 = os.environ.get("DEC_BF16", "0") == "1"
    np16 = ml_dtypes.bfloat16 if USE_BF16 else np.float16

    skip = int(skip_size)
    g_idx, g_zero, p_idx, p_zero = _skip_tables(T, skip)
    a0 = np.asarray(mask0).astype(np.int64) != 0
    a1 = np.asarray(mask1).astype(np.int64) != 0

    b_ih = np.asarray(b_ih, np.float32)
    b_hh = np.asarray(b_hh, np.float32)
    b_out = np.asarray(b_out, np.float32)
    rz_bias = (b_ih + b_hh)[0:512]
    inn_bias = b_ih[512:768]
    hn_bias = b_hh[512:768]
    rz_nz = bool(np.any(rz_bias != 0))
    inn_nz = bool(np.any(inn_bias != 0))
    hn_nz = bool(np.any(hn_bias != 0))
    bout_nz = bool(np.any(b_out != 0))

    key = (T, a0.tobytes(), a1.tobytes(), skip, rz_nz, inn_nz, hn_nz, bout_nz,
           USE_BF16)
    nc = _get_program(key, T, a0, a1, g_idx, g_zero, p_idx, p_zero,
                      rz_nz, inn_nz, hn_nz, bout_nz, USE_BF16)

    W_ih = np.asarray(W_ih, np.float32)
    W_hh = np.asarray(W_hh, np.float32)
    W_out = np.asarray(W_out, np.float32)
    h_enc = np.asarray(h_enc, np.float32)

    wih_h = np.ascontiguousarray(W_ih.T).astype(np16)          # [64, 768]
    whhT = W_hh.T.astype(np16)                                  # [256, 768]
    whh_h = np.ascontiguousarray(np.concatenate(
        [whhT[0:128, :], whhT[128:256, :]], axis=1))                  # [128, 1536]
    woutT = W_out.T.astype(np16)                                # [256, 64]
    wout_h = np.ascontiguousarray(np.concatenate(
        [woutT[0:128, :], woutT[128:256, :]], axis=1))                # [128, 128]
    ident_h = np.eye(64, dtype=np16)
    bias_h = np.zeros((1, 1024), np16)
    bias_h[0, 0:512] = rz_bias.astype(np16)
    bias_h[0, 512:768] = inn_bias.astype(np16)
    bias_h[0, 768:1024] = hn_bias.astype(np16)
    bout_h = b_out.reshape(64, 1).astype(np.float32)

    in_maps = []
    for c in range(N_CORES):
        hc = h_enc[c * BL:(c + 1) * BL]                               # [16, 256]
        henc_h = np.ascontiguousarray(
            hc.T.reshape(2, 128, BL).transpose(1, 0, 2).reshape(128, 2 * BL)
        ).astype(np16)
        in_maps.append({
            "henc": henc_h,
            "wih": wih_h,
            "whh": whh_h,
            "wout": wout_h,
            "ident": ident_h,
            "biases": bias_h,
            "bout": bout_h,
        })

    res = run_bass_kernel_spmd(nc, in_maps, core_ids=list(range(N_CORES)),
                               trace=trace)
    out = np.concatenate([r["out"] for r in res.results], axis=0)
    return out.astype(np.float32), res


def kernel(**inputs):
    out, _ = kernel_with_results(**inputs)
    return out


# revision 19
# speedup vs baseline: 1.2848x; 1.1240x over previous
"""Trainium2 Bass kernel for nn_Decoder (autoregressive GRU decoder with skip
connections).

Strategy:
  - Data-parallel over batch: B=128 -> 8 cores x 16. No collectives; every
    core runs an identical program on its own batch slice.
  - The `input` tensor's VALUES are unused by the reference (the GRU consumes
    its own previous projected output), so it is never transferred.
  - mask0/mask1/skip_size are host-visible at build time; the T=512 step loop
    is fully unrolled and specialized per step (the masked adds become tile
    aliasing / a single add / a zero tile).
  - Layout: states are feature-on-partition, fp16. A hidden-state slot is
    [128 partitions, 2*16] (H=256 split into 2 chunks of 128, batch=16 on the
    free axis). The full GRU-output history buf lives in SBUF ([128, 512*32]
    fp16 = 32KB/partition), as does the projected-output history.
  - Matmuls: weights stationary (fp16 => fast-weight-load), states moving
    (N=16). gi and gh accumulate into the same PSUM tile so ir+hr / iz+hz
    need no extra adds. n-gate keeps inn and hn in separate PSUM columns.
  - Output projection feeds both the output history and the next step's GRU
    input, so it is computed once per step into [X=64, B] layout; the final
    [B, T, X] transpose is done at the end with 64 PE-transposes.
"""

import os
import numpy as np

B, T, X, H = 128, 512, 64, 256
N_CORES = 8
BL = B // N_CORES  # 16 batch per core
NG = 6             # gate chunks of 128 (r0 r1 z0 z1 n0 n1)


def _skip_tables(T, skip):
    # replicates the reference's Python-list negative indexing
    g_idx = np.zeros(T, np.int64); g_zero = np.zeros(T, bool)
    p_idx = np.zeros(T, np.int64); p_zero = np.zeros(T, bool)
    for i in range(T):
        pg = 2 * i if i < skip else i - skip
        pp = 2 * i + 1 if i < skip else i - skip
        g_zero[i] = pg < skip; g_idx[i] = max(pg - skip, 0)
        p_zero[i] = pp < skip; p_idx[i] = max(pp - skip, 0)
    return g_idx, g_zero, p_idx, p_zero


def _build_program(n_steps, a0, a1, g_idx, g_zero, p_idx, p_zero,
                   rz_bias_nz, inn_bias_nz, hn_bias_nz, out_bias_nz,
                   use_bf16=False):
    import concourse.bass as bass  # noqa: F401
    import concourse.bacc as bacc
    import concourse.tile as tile
    from concourse import mybir
    from concourse.tile_rust import add_dep_helper
    from contextlib import ExitStack

    f16 = mybir.dt.bfloat16 if use_bf16 else mybir.dt.float16
    f32 = mybir.dt.float32
    AF = mybir.ActivationFunctionType
    OP = mybir.AluOpType

    any_bias = rz_bias_nz or inn_bias_nz or hn_bias_nz

    nc = bacc.Bacc("TRN2", target_bir_lowering=False, debug=False)

    henc_d = nc.dram_tensor("henc", [128, 2 * BL], f16, kind="ExternalInput")
    wih_d = nc.dram_tensor("wih", [64, 768], f16, kind="ExternalInput")
    whh_d = nc.dram_tensor("whh", [128, 2 * 768], f16, kind="ExternalInput")
    wout_d = nc.dram_tensor("wout", [128, 2 * 64], f16, kind="ExternalInput")
    ident_d = nc.dram_tensor("ident", [64, 64], f16, kind="ExternalInput")
    bias_d = nc.dram_tensor("biases", [1, 1024], f16, kind="ExternalInput")
    bout_d = nc.dram_tensor("bout", [64, 1], f32, kind="ExternalInput")
    out_d = nc.dram_tensor("out", [BL, T, X], f32, kind="ExternalOutput")

    with tile.TileContext(nc) as tc, ExitStack() as ctx:
        const = ctx.enter_context(tc.tile_pool(name="const", bufs=1))
        big = ctx.enter_context(tc.tile_pool(name="big", bufs=1))

        wih = const.tile([64, 768], f16)
        nc.sync.dma_start(wih[:, :], wih_d[:, :])
        whh = const.tile([128, 2 * 768], f16)
        nc.sync.dma_start(whh[:, :], whh_d[:, :])
        wout = const.tile([128, 2 * 64], f16)
        nc.sync.dma_start(wout[:, :], wout_d[:, :])
        ident = const.tile([64, 64], f16)
        nc.sync.dma_start(ident[:, :], ident_d[:, :])
        henc = const.tile([128, 2 * BL], f16)
        nc.sync.dma_start(henc[:, :], henc_d[:, :])
        zero = const.tile([128, 2 * BL], f16)
        nc.vector.memset(zero[:, :], 0.0)
        bias = const.tile([1, 1024], f16)
        ones = const.tile([1, BL], f16)
        bout = const.tile([64, 1], f32)
        if any_bias:
            nc.sync.dma_start(bias[:, :], bias_d[:, :])
            nc.vector.memset(ones[:, :], 1.0)
        if out_bias_nz:
            nc.sync.dma_start(bout[:, :], bout_d[:, :])

        # GRU output history: slot i at cols [i*32, (i+1)*32); chunk c at
        # [i*32 + c*16, ...). Also the h state itself (h_{i} == slot i).
        buf = big.tile([128, T * 2 * BL], f16)
        # projected-output history: slot t holds x_t (the GRU input at step
        # t); slot t+1 is written by step t's projection. Slot 0 is the GO
        # token (zeros).
        xh = big.tile([64, (T + 1) * BL], f16)
        nc.vector.memset(xh[:, 0:BL], 0.0)

        def slot(i):
            if i < 0:
                return henc[:, :]
            return buf[:, i * 2 * BL:(i + 1) * 2 * BL]

        with (
            tc.tile_pool(name="work", bufs=8) as work,
            tc.tile_pool(name="psum", bufs=2, space="PSUM") as psum,
            tc.tile_pool(name="psumx", bufs=2, space="PSUM") as psumx,
        ):
            for t in range(n_steps):
                # ---- resolve hidden = a0*h_prev + a1*skip_g ----
                terms = []
                if a0[t]:
                    terms.append(t - 1)
                if a1[t] and not g_zero[t] and g_idx[t] < t:
                    terms.append(int(g_idx[t]))
                if len(terms) == 0:
                    hid = zero[:, :]
                    hid_zero = True
                elif len(terms) == 1:
                    hid = slot(terms[0])
                    hid_zero = False
                else:
                    hidt = work.tile([128, 2 * BL], f16, tag="hid")
                    nc.vector.tensor_add(hidt[:, :], slot(terms[0]), slot(terms[1]))
                    hid = hidt[:, :]
                    hid_zero = False

                has_gi = t > 0
                has_gh = not hid_zero

                tanh_inst = None
                if not has_gi and not has_gh and not any_bias:
                    # gates see all-zero inputs: r=z=0.5, n=tanh(0)=0
                    # => h_new = 0.5*0 + 0.5*0 = 0
                    nc.vector.memset(slot(t), 0.0)
                else:
                    # ---- gate matmuls ----
                    # separate PSUM tiles (= banks) for r / z / n so the
                    # r-sigmoid is not bank-serialized behind z/proj writes
                    ps_r = psum.tile([128, 2 * BL], f32, tag="r")
                    ps_z = psum.tile([128, 2 * BL], f32, tag="z")
                    # n psum: cols 0:32 = inn chunks, 32:64 = hn chunks
                    ps_n = psum.tile([128, 4 * BL], f32, tag="n")
                    xrhs = xh[:, t * BL:(t + 1) * BL]

                    def gate_group(out_ap, gc, with_gi, with_gh, bias_col):
                        mms = []
                        if with_gh:
                            mms.append(("gh", 0))
                            mms.append(("gh", 1))
                        if with_gi:
                            mms.append(("gi", 0))
                        if bias_col is not None:
                            mms.append(("bias", bias_col))
                        if not mms:
                            nc.vector.memset(out_ap, 0.0)
                            return
                        for j, (kind, k) in enumerate(mms):
                            st = j == 0
                            sp = j == len(mms) - 1
                            if kind == "gh":
                                nc.tensor.matmul(
                                    out_ap,
                                    whh[:, k * 768 + gc * 128:k * 768 + (gc + 1) * 128],
                                    hid[:, k * BL:(k + 1) * BL],
                                    start=st, stop=sp)
                            elif kind == "gi":
                                nc.tensor.matmul(
                                    out_ap,
                                    wih[:, gc * 128:(gc + 1) * 128],
                                    xrhs,
                                    start=st, stop=sp)
                            else:
                                nc.tensor.matmul(
                                    out_ap,
                                    bias[:, k:k + 128],
                                    ones[:, :],
                                    start=st, stop=sp)

                    # critical-path-first PE order: r gates, hn, inn, then z
                    for g in range(2):  # r0 r1
                        bc = g * 128 if rz_bias_nz else None
                        gate_group(ps_r[:, g * BL:(g + 1) * BL], g, has_gi,
                                   has_gh, bc)
                    if has_gh or hn_bias_nz:
                        for c in range(2):  # hn chunks (gh only)
                            bc = 768 + c * 128 if hn_bias_nz else None
                            gate_group(ps_n[:, (2 + c) * BL:(3 + c) * BL],
                                       4 + c, False, has_gh, bc)
                    if has_gi or inn_bias_nz:
                        for c in range(2):  # inn chunks (gi only)
                            bc = 512 + c * 128 if inn_bias_nz else None
                            gate_group(ps_n[:, c * BL:(c + 1) * BL], 4 + c,
                                       has_gi, False, bc)
                    for g in range(2, 4):  # z0 z1
                        bc = g * 128 if rz_bias_nz else None
                        gate_group(ps_z[:, (g - 2) * BL:(g - 1) * BL], g,
                                   has_gi, has_gh, bc)

                    # ---- gates ----
                    rz = work.tile([128, 4 * BL], f32, tag="rz_sb")
                    r_ap = rz[:, 0:2 * BL]
                    z_ap = rz[:, 2 * BL:4 * BL]
                    # r sigmoid is on the critical path; z is hidden under
                    # the n-path
                    nc.scalar.activation(r_ap, ps_r[:, :], AF.Sigmoid)
                    nc.scalar.activation(z_ap, ps_z[:, :], AF.Sigmoid)

                    # off-critical-path work on gpsimd
                    if hid_zero:
                        omz = work.tile([128, 2 * BL], f16, tag="omz")
                        nc.gpsimd.tensor_scalar(omz[:, :], z_ap, -1.0, 1.0,
                                                OP.mult, OP.add)
                    else:
                        zh = work.tile([128, 2 * BL], f16, tag="zh")
                        nc.gpsimd.tensor_mul(zh[:, :], z_ap, hid)

                    # n = tanh(inn + r*hn)
                    if has_gh or hn_bias_nz:
                        rhn = work.tile([128, 2 * BL], f32, tag="rhn")
                        nc.vector.tensor_mul(rhn[:, :], r_ap,
                                             ps_n[:, 2 * BL:4 * BL])
                        if has_gi or inn_bias_nz:
                            npre = work.tile([128, 2 * BL], f32, tag="npre")
                            nc.vector.tensor_add(npre[:, :], rhn[:, :],
                                                 ps_n[:, 0:2 * BL])
                            n_src = npre[:, :]
                        else:
                            n_src = rhn[:, :]
                    else:
                        n_src = ps_n[:, 0:2 * BL]
                    n_sb = work.tile([128, 2 * BL], f16, tag="n_sb")
                    tanh_inst = nc.scalar.activation(n_sb[:, :], n_src, AF.Tanh)

                    # ---- h_new = (1-z)*n + z*hidden -> buf[t] ----
                    if hid_zero:
                        nc.vector.tensor_mul(slot(t), omz[:, :], n_sb[:, :])
                    else:
                        # t1 = (z-1)*n in one fused DVE op (no omz needed),
                        # then h_new = zh - t1
                        t1 = work.tile([128, 2 * BL], f16, tag="t1")
                        nc.vector.scalar_tensor_tensor(
                            t1[:, :], z_ap, 1.0, n_sb[:, :],
                            OP.subtract, OP.mult)
                        nc.vector.tensor_sub(slot(t), zh[:, :], t1[:, :])

                # ---- projection out_t = (h_prev + skip_p) @ W_out^T ----
                # projection input (h_prev + skip_p) is fed as multiple
                # moving operands accumulated in PSUM - no explicit add
                pterms = [t - 1]
                if not p_zero[t] and p_idx[t] <= t:
                    pterms.append(int(p_idx[t]))
                ps_x = psumx.tile([64, BL], f32, tag="xp")
                n_mm = 2 * len(pterms)
                j = 0
                for pt in pterms:
                    src = slot(pt)
                    for k in range(2):
                        nc.tensor.matmul(ps_x[:, :],
                                         wout[:, k * 64:(k + 1) * 64],
                                         src[:, k * BL:(k + 1) * BL],
                                         start=(j == 0), stop=(j == n_mm - 1))
                        j += 1
                xdst = xh[:, (t + 1) * BL:(t + 2) * BL]
                if out_bias_nz:
                    cast_inst = nc.scalar.activation(xdst, ps_x[:, :],
                                                     AF.Identity,
                                                     bias=bout[:, :])
                else:
                    cast_inst = nc.vector.tensor_copy(xdst, ps_x[:, :])
                if tanh_inst is not None:
                    # keep the x-history cast behind this step's chain ops in
                    # the DVE stream so it cannot head-of-line block them
                    add_dep_helper(cast_inst.ins, tanh_inst.ins, sync=False,
                                   reason="xcast after chain")

        # ---- end phase: transpose xh slots 1..n_steps into out [BL, T, X] ----
        xh3 = xh[:, :].rearrange("p (s b) -> p s b", b=BL)
        n_tc = (n_steps + 127) // 128
        with (
            tc.tile_pool(name="endsb", bufs=2) as endsb,
            tc.tile_pool(name="endps", bufs=4, space="PSUM") as endps,
        ):
            for bt in range(BL):
                sbv = endsb.tile([128, n_tc * 64], f32, tag="osb")
                for c in range(n_tc):
                    pst = endps.tile([128, 64], f16, tag="pst")
                    lo = 1 + c * 128
                    nsl = min(128, n_steps - c * 128)
                    nc.tensor.transpose(pst[0:nsl, :],
                                        xh3[:, lo:lo + nsl, bt], ident[:, :])
                    nc.scalar.activation(sbv[0:nsl, c * 64:(c + 1) * 64],
                                         pst[0:nsl, :], AF.Copy)
                for c in range(n_tc):
                    nsl = min(128, n_steps - c * 128)
                    nc.sync.dma_start(
                        out=out_d[bt, c * 128:c * 128 + nsl, :],
                        in_=sbv[0:nsl, c * 64:(c + 1) * 64])
    nc.compile()
    return nc


_PROG_CACHE = {}


def _get_program(key, *args):
    if key not in _PROG_CACHE:
        _PROG_CACHE[key] = _build_program(*args)
    return _PROG_CACHE[key]


def kernel_with_results(input, h_enc, W_ih, W_hh, b_ih, b_hh, W_out, b_out,
                        mask0, mask1, skip_size, trace=False):
    from concourse.bass_utils import run_bass_kernel_spmd
    import ml_dtypes

    USE_BF16 = os.environ.get("DEC_BF16", "0") == "1"
    np16 = ml_dtypes.bfloat16 if USE_BF16 else np.float16

    skip = int(skip_size)
    g_idx, g_zero, p_idx, p_zero = _skip_tables(T, skip)
    a0 = np.asarray(mask0).astype(np.int64) != 0
    a1 = np.asarray(mask1).astype(np.int64) != 0

    b_ih = np.asarray(b_ih, np.float32)
    b_hh = np.asarray(b_hh, np.float32)
    b_out = np.asarray(b_out, np.float32)
    rz_bias = (b_ih + b_hh)[0:512]
    inn_bias = b_ih[512:768]
    hn_bias = b_hh[512:768]
    rz_nz = bool(np.any(rz_bias != 0))
    inn_nz = bool(np.any(inn_bias != 0))
    hn_nz = bool(np.any(hn_bias != 0))
    bout_nz = bool(np.any(b_out != 0))

    key = (T, a0.tobytes(), a1.tobytes(), skip, rz_nz, inn_nz, hn_nz, bout_nz,
           USE_BF16)
    nc = _get_program(key, T, a0, a1, g_idx, g_zero, p_idx, p_zero,
                      rz_nz, inn_nz, hn_nz, bout_nz, USE_BF16)

    W_ih = np.asarray(W_ih, np.float32)
    W_hh = np.asarray(W_hh, np.float32)
    W_out = np.asarray(W_out, np.float32)
    h_enc = np.asarray(h_enc, np.float32)

    wih_h = np.ascontiguousarray(W_ih.T).astype(np16)          # [64, 768]
    whhT = W_hh.T.astype(np16)                                  # [256, 768]
    whh_h = np.ascontiguousarray(np.concatenate(
        [whhT[0:128, :], whhT[128:256, :]], axis=1))                  # [128, 1536]
    woutT = W_out.T.astype(np16)                                # [256, 64]
    wout_h = np.ascontiguousarray(np.concatenate(
        [woutT[0:128, :], woutT[128:256, :]], axis=1))                # [128, 128]
    ident_h = np.eye(64, dtype=np16)
    bias_h = np.zeros((1, 1024), np16)
    bias_h[0, 0:512] = rz_bias.astype(np16)
    bias_h[0, 512:768] = inn_bias.astype(np16)
    bias_h[0, 768:1024] = hn_bias.astype(np16)
    bout_h = b_out.reshape(64, 1).astype(np.float32)

    in_maps = []
    for c in range(N_CORES):
        hc = h_enc[c * BL:(c + 1) * BL]                               # [16, 256]
        henc_h = np.ascontiguousarray(
            hc.T.reshape(2, 128, BL).transpose(1, 0, 2).reshape(128, 2 * BL)
        ).astype(np16)
        in_maps.append({
            "henc": henc_h,
            "wih": wih_h,
            "whh": whh_h,
            "wout": wout_h,
            "ident": ident_h,
            "biases": bias_h,
            "bout": bout_h,
        })

    res = run_bass_kernel_spmd(nc, in_maps, core_ids=list(range(N_CORES)),
                               trace=trace)
    out = np.concatenate([r["out"] for r in res.results], axis=0)
    return out.astype(np.float32), res


def kernel(**inputs):
    out, _ = kernel_with_results(**inputs)
    return out
